# revision 14
# baseline (speedup 1.0000x reference)
"""Trainium2 Bass kernel for nn_HGNN_ATT (HGNN message passing, K sub-graphs).

v2 design:
- Nodes re-permuted so every shard holds an equal user/item mix; SHP=12800
  rows per core = 100 windows of 128 = 4 slices of 3200 rows.
- Global gather table is slice-major: g = s*8*SLR + m*SLR + r, so each of
  the 4 table slices is its own DRAM tensor (25600 rows, int16-indexable)
  and its own AllGather -> per-slice collective/compute pipelining.
- D^-1/2 A D^-1/2 folded into the tables: table rows pre-scaled by dinv,
  psum results post-scaled by dinv.  No per-edge `val` multiply at all.
- Edges sorted by (tgt slice, window-group, src bucket, window, gather row).
  Gather chunks are (s, wg, b) runs; matmuls consume window-major across
  the 4 buckets so each window accumulates in a single PSUM bank across
  its whole contraction (one psum->SBUF copy per window).
- One-hot S tiles built on DVE via is_equal(srel, iota) in int16; padded
  slots have srel=-1 so they vanish.  Per slice, all gather indices and
  srel values arrive in ONE packed int16 "meta" DMA (gidx pre-replicated
  to 128 partitions).
"""

import os

import numpy as np

import concourse.bass as bass
import concourse.mybir as mybir
import concourse.tile as tile
from concourse import bacc
from concourse.masks import make_identity

F32 = mybir.dt.float32
I16 = mybir.dt.int16
I32 = mybir.dt.int32
AF = mybir.ActivationFunctionType

NCORES = 8
D = 64
P = 128
NSL = 4                  # table slices (= gather buckets = AllGather units)
WG = 4                   # windows per concurrently-open PSUM group set
GCAP = 24                # max tiles per gather chunk (G tile free dim)
SCAP = 8                 # max tiles per one-hot build
CCH = 512                # post-phase column chunk
SKIP_AG = bool(int(os.environ.get("SKIP_AG", "0")))     # timing experiment
SKIP_GATHER = bool(int(os.environ.get("SKIP_GATHER", "0")))
SKIP_MM = bool(int(os.environ.get("SKIP_MM", "0")))


class Cfg:
    def __init__(self, NU, NI, K, E):
        assert NU % NCORES == 0 and NI % NCORES == 0
        self.NU, self.NI, self.K, self.E = NU, NI, K, E
        self.UPC = NU // NCORES
        self.IPC = NI // NCORES
        self.SH = self.UPC + self.IPC              # real rows per shard
        self.SHP = ((self.SH + NSL * P - 1) // (NSL * P)) * (NSL * P)
        self.SLR = self.SHP // NSL                 # rows per slice per core
        self.NT = self.SHP // P                    # windows per shard
        self.SLW = self.SLR // P                   # windows per slice
        # gather bucket: 2 cores' worth of rows (int16-indexable)
        self.BUCKET = 2 * self.SHP
        assert self.BUCKET <= 32768 and (NCORES * self.SHP) % self.BUCKET == 0


def _perm_maps(cfg):
    """original node id -> (core, local_row)."""
    orig = np.arange(cfg.NU + cfg.NI)
    is_item = orig >= cfg.NU
    core = np.where(is_item, (orig - cfg.NU) // cfg.IPC, orig // cfg.UPC)
    loc = np.where(is_item, cfg.UPC + (orig - cfg.NU) % cfg.IPC,
                   orig % cfg.UPC)
    return core.astype(np.int64), loc.astype(np.int64)


def _wgroups(cfg):
    """window-group (start, size) list per slice, e.g. SLW=25 ->
    [(0,4),(4,4),...,(24,1)]."""
    out = []
    w = 0
    while w < cfg.SLW:
        n = min(WG, cfg.SLW - w)
        out.append((w, n))
        w += n
    return out


def prep(cfg, rows, cols):
    """Host-side graph preprocessing.

    Returns (plan, per_core).  plan[k] holds the shared (max-over-cores)
    chunk/tile structure; per_core holds the packed meta and dinv arrays.
    """
    NU, NI, K = cfg.NU, cfg.NI, cfg.K
    N = NU + NI
    core_of, loc_of = _perm_maps(cfg)
    wgroups = _wgroups(cfg)
    NWG = len(wgroups)

    plan = []
    meta_cols = [[] for _ in range(NCORES)]
    dinv_arr = np.zeros((NCORES, K, cfg.SHP), np.float32)

    for k in range(K):
        r = np.asarray(rows[k]).astype(np.int64)
        c = np.asarray(cols[k]).astype(np.int64)
        src = np.concatenate([r, c + NU])
        dst = np.concatenate([c + NU, r])
        deg = np.bincount(src, minlength=N).astype(np.float32) \
            + np.float32(1e-7)
        dinv = (deg ** np.float32(-0.5)).astype(np.float32)
        for m in range(NCORES):
            sel = core_of == m
            dinv_arr[m, k, loc_of[sel]] = dinv[sel]

        e_core = core_of[src]
        e_tloc = loc_of[src]                   # accumulation target row
        d_core = core_of[dst]
        d_loc = loc_of[dst]
        g_glob = d_core * cfg.SHP + d_loc      # core-major global table row
        e_b = g_glob // cfg.BUCKET             # gather bucket
        e_grow = g_glob % cfg.BUCKET           # row within bucket
        e_w = e_tloc // P                      # target window (global)
        e_sl = e_w // cfg.SLW                  # target slice
        e_wrel = e_w % cfg.SLW
        wg_cum = np.cumsum([n for (_, n) in wgroups])
        e_wg = np.searchsorted(wg_cum, e_wrel, side="right")

        cell_id = (((e_core * NSL + e_sl) * NWG + e_wg) * NSL + e_b) \
            * cfg.NT + e_w
        ncell_m = NSL * NWG * NSL * cfg.NT
        cnt = np.bincount(cell_id, minlength=NCORES * ncell_m).reshape(
            NCORES, NSL, NWG, NSL, cfg.NT)
        nt = np.ceil(cnt.max(axis=0) / P).astype(np.int64)
        # nt: [NSL(tgt slice), NWG, NSL(bucket), NT(global w)]

        # chunk structure shared by all cores: chunks[s*NWG+wg] =
        # [(b, [w,...tiles]), ...] with each tile list <= GCAP
        chunks = []
        for s in range(NSL):
            for gi_, (w0, nw) in enumerate(wgroups):
                ch_list = []
                for b in range(NSL):
                    tl = []
                    for w in range(s * cfg.SLW + w0,
                                   s * cfg.SLW + w0 + nw):
                        tl += [w] * int(nt[s, gi_, b, w])
                    for o in range(0, len(tl), GCAP):
                        ch_list.append((b, tl[o:o + GCAP]))
                chunks.append(ch_list)
        plan.append({"nt": nt, "chunks": chunks, "wgroups": wgroups})

        # per-core packed meta, ordered identically to chunks
        order = np.lexsort((e_grow, e_w, e_b, e_wg, e_sl, e_core))
        s_core = e_core[order]
        s_tloc = e_tloc[order]
        s_grow = e_grow[order]
        key_all = cell_id[order]
        start = np.searchsorted(s_core, np.arange(NCORES))
        stop = np.searchsorted(s_core, np.arange(NCORES) + 1)

        for m in range(NCORES):
            mk = key_all[start[m]:stop[m]] - m * ncell_m
            mg = s_grow[start[m]:stop[m]]
            mt = s_tloc[start[m]:stop[m]]
            cs = np.searchsorted(mk, np.arange(ncell_m))
            ce = np.searchsorted(mk, np.arange(ncell_m) + 1)
            gi_parts, sr_parts = [], []
            for s in range(NSL):
                for gi_, (w0, nw) in enumerate(wgroups):
                    for b in range(NSL):
                        for w in range(s * cfg.SLW + w0,
                                       s * cfg.SLW + w0 + nw):
                            npad = int(nt[s, gi_, b, w]) * P
                            if npad == 0:
                                continue
                            cid = ((s * NWG + gi_) * NSL + b) * cfg.NT + w
                            a, z = cs[cid], ce[cid]
                            n = z - a
                            gi_buf = np.zeros(npad, np.int64)
                            sr_buf = np.full(npad, -1, np.int64)
                            gi_buf[:n] = mg[a:z]
                            sr_buf[:n] = mt[a:z] - w * P
                            gi_parts.append(gi_buf)
                            sr_parts.append(sr_buf)
            gi_all = np.concatenate(gi_parts) if gi_parts else \
                np.zeros(0, np.int64)
            sr_all = np.concatenate(sr_parts) if sr_parts else \
                np.zeros(0, np.int64)
            off = 0
            for swg in range(NSL * NWG):
                for (b, tl) in plan[k]["chunks"][swg]:
                    ntile = len(tl)
                    L = ntile * P
                    seg_g = gi_all[off:off + L].astype(np.int16)
                    seg16 = seg_g.reshape(L // 16, 16).T      # [16, L/16]
                    meta_cols[m].append(np.tile(seg16, (8, 1)))
                    sr16 = sr_all[off:off + L].astype(np.int16) \
                        .reshape(ntile, P).T                  # [128, ntile]
                    meta_cols[m].append(sr16)
                    off += L
            assert off == len(gi_all)

    per_core = []
    for m in range(NCORES):
        dv = dinv_arr[m].reshape(cfg.K, cfg.NT, P).transpose(2, 0, 1) \
            .reshape(P, cfg.K * cfg.NT)
        per_core.append({
            "meta": np.ascontiguousarray(
                np.concatenate(meta_cols[m], axis=1)),
            "dinv": np.ascontiguousarray(dv),
        })
    return plan, per_core


def _wg_meta_cols(cfg, plan, k, s, gi_):
    wgroups = plan[k]["wgroups"]
    return sum(len(tl) * 9
               for (b, tl) in plan[k]["chunks"][s * len(wgroups) + gi_])


def build(cfg, plan):
    nc = bacc.Bacc("TRN2", target_bir_lowering=False, debug=False,
                   num_devices=NCORES, num_swdge_queues=4,
                   dynamic_dma_scratch_size=65536)
    K, SHP, NT, SLR, SLW = cfg.K, cfg.SHP, cfg.NT, cfg.SLR, cfg.SLW
    BUCKET = cfg.BUCKET
    wgroups = _wgroups(cfg)
    NWG = len(wgroups)
    TOTM = sum(_wg_meta_cols(cfg, plan, k, s, g)
               for k in range(K) for s in range(NSL) for g in range(NWG))
    SMAX = max(_wg_meta_cols(cfg, plan, k, s, g)
               for k in range(K) for s in range(NSL) for g in range(NWG))

    xT_in = nc.declare_dram_parameter("xT", [D, SHP], F32, isOutput=False)
    biascol = nc.declare_dram_parameter("biascol", [D, 1], F32,
                                        isOutput=False)
    fc1_WT = nc.declare_dram_parameter("fc1_WT", [D, D], F32, isOutput=False)
    fus1_WT = nc.declare_dram_parameter("fus1_WT", [D, D], F32,
                                        isOutput=False)
    b1col = nc.declare_dram_parameter("b1col", [D, 1], F32, isOutput=False)
    w2col = nc.declare_dram_parameter("w2col", [D, 1], F32, isOutput=False)
    meta_d = nc.declare_dram_parameter("meta", [P, TOTM], I16,
                                       isOutput=False)
    dinv_d = nc.declare_dram_parameter("dinv", [P, K * NT], F32,
                                       isOutput=False)

    nodesT_o = nc.declare_dram_parameter("nodesT", [K, D, SHP], F32,
                                         isOutput=True)
    edges_o = nc.declare_dram_parameter("edges", [K, SHP, D], F32,
                                        isOutput=True)

    h_bounce = nc.dram_tensor("h_bounce", [SHP, D], F32)
    e_bounce = nc.dram_tensor("e_bounce", [SHP, D], F32)
    h_full = nc.dram_tensor("h_full", [NCORES * SHP, D], F32,
                            addr_space="Shared")
    e_full = nc.dram_tensor("e_full", [NCORES * SHP, D], F32,
                            addr_space="Shared")
    xT_pp = [nc.dram_tensor("xT_a", [D, SHP], F32),
             nc.dram_tensor("xT_b", [D, SHP], F32)]

    RG = [list(range(NCORES))]

    with tile.TileContext(nc) as tc:
        with tc.tile_pool(name="persist", bufs=1) as pp, \
             tc.tile_pool(name="gpool", bufs=9) as gp, \
             tc.tile_pool(name="spool", bufs=5) as sp, \
             tc.tile_pool(name="mpool", bufs=3) as mp, \
             tc.tile_pool(name="stg", bufs=2) as stg, \
             tc.tile_pool(name="stg1", bufs=1) as stg1, \
             tc.tile_pool(name="post", bufs=2) as po, \
             tc.tile_pool(name="pswin", bufs=3, space="PSUM") as psw, \
             tc.tile_pool(name="pspost", bufs=3, space="PSUM") as psp, \
             tc.tile_pool(name="psh", bufs=2, space="PSUM") as psh:

            ident = pp.tile([P, P], F32)
            make_identity(nc, ident[:])
            iota_i = pp.tile([P, P], I32)
            nc.gpsimd.iota(iota_i[:], pattern=[[1, P]], base=0,
                           channel_multiplier=0)
            iota16 = pp.tile([P, P], I16)
            nc.vector.tensor_copy(iota16[:], iota_i[:])
            acc = pp.tile([P, NT * D], F32)
            wfc1 = pp.tile([D, D], F32)
            nc.sync.dma_start(wfc1[:], fc1_WT[:, :])
            wfus = pp.tile([D, D], F32)
            nc.sync.dma_start(wfus[:], fus1_WT[:, :])
            bcol = pp.tile([D, 1], F32)
            nc.sync.dma_start(bcol[:], biascol[:, :])
            b1c = pp.tile([D, 1], F32)
            nc.sync.dma_start(b1c[:], b1col[:, :])
            w2c = pp.tile([D, 1], F32)
            nc.sync.dma_start(w2c[:], w2col[:, :])
            ones1 = pp.tile([1, D], F32)
            nc.vector.memset(ones1[:], 1.0)
            iota16f_dummy = pp.tile([P, P], F32)
            nc.vector.memset(iota16f_dummy[:], 0.5)
            dinva = pp.tile([P, K * NT], F32)
            nc.sync.dma_start(dinva[:], dinv_d[:, :])

            nc.sync.dma_start(xT_pp[0][:, :], xT_in[:, :])

            qctr = [0]
            mo = [0]   # running column offset into meta_d
            # dummy completion sem for preps: stripped right after emit so
            # Tile's sem-assignment attaches its own DMASW lane sem instead
            gsem_dummy = nc.alloc_semaphore("gsem_dummy")

            def spmm(k, phase, table):
                """phase 1: h->edge (writes edges_o, e_bounce, AG e);
                   phase 2: e->node (writes acc)."""
                dv = dinva[:, k * NT:(k + 1) * NT]
                for s in range(NSL):
                    if phase == 1:
                        stgE = stg.tile([P, SLW * D], F32, tag="stgE")
                        stgT = stg1.tile([P, SLW * D], F32, tag="stgT")
                    for gi_, (w0, nw) in enumerate(wgroups):
                        ch_list = plan[k]["chunks"][s * NWG + gi_]
                        mcols = _wg_meta_cols(cfg, plan, k, s, gi_)
                        meta_t = mp.tile([P, SMAX], I16, tag="meta")
                        if mcols > 0:
                            nc.sync.dma_start(meta_t[:, 0:mcols],
                                              meta_d[:, mo[0]:mo[0] + mcols])
                        mo[0] += mcols
                        moff = 0
                        g_tiles = []
                        sr_offs = []
                        for (b, tl) in ch_list:
                            ntile = len(tl)
                            L = ntile * P
                            G = gp.tile([P, GCAP, D], F32, tag="G")
                            q = qctr[0] % 4
                            if SKIP_GATHER:
                                nc.vector.memset(G[:, 0:ntile, :], 0.01)
                            else:
                                bi = nc.gpsimd.dma_gather(
                                    out_ap=G[:, 0:ntile, :],
                                    in_ap=table[b * BUCKET:(b + 1) * BUCKET,
                                                :],
                                    idxs_ap=meta_t[:, moff:moff + ntile * 8],
                                    num_idxs=L, num_idxs_reg=L, elem_size=D,
                                    single_packet=False, prepare_only=True,
                                    sem=gsem_dummy, queue_num=q)
                                bi.ins.sync_info.on_update = []
                                nc.gpsimd.trigger_dma(count=None,
                                                      queue_num=q)
                            qctr[0] += 1
                            g_tiles.append(G)
                            sr_offs.append(moff + ntile * 8)
                            moff += ntile * 9
                        # window -> [(chunk, pos)] map
                        wlist = list(range(s * SLW + w0, s * SLW + w0 + nw))
                        wmap = {w: [] for w in wlist}
                        for ci, (b, tl) in enumerate(ch_list):
                            for pos, w in enumerate(tl):
                                wmap[w].append((ci, pos))
                        for w in wlist:
                            tiles = wmap[w]
                            if not tiles:
                                if phase == 1:
                                    wr = w - s * SLW
                                    nc.vector.memset(
                                        stgE[:, wr * D:(wr + 1) * D], 0.0)
                                else:
                                    nc.vector.memset(
                                        acc[:, w * D:(w + 1) * D], 0.0)
                                continue
                            ps = psw.tile([P, 512], F32, tag="ps")
                            if SKIP_MM:
                                nc.tensor.matmul(
                                    out=ps[:, 0:D],
                                    lhsT=iota16f_dummy[:, 0:P],
                                    rhs=iota16f_dummy[:, 0:D],
                                    start=True, stop=True)
                            # contiguous runs of tiles within one chunk
                            runs = []
                            for (ci, pos) in tiles:
                                if runs and runs[-1][0] == ci and \
                                        pos == runs[-1][1] + runs[-1][2] \
                                        and runs[-1][2] < SCAP:
                                    runs[-1][2] += 1
                                else:
                                    runs.append([ci, pos, 1])
                            tcount = 0
                            ntot = len(tiles)
                            for (ci, pos, n) in (runs if not SKIP_MM
                                                 else []):
                                S = sp.tile([P, SCAP, P], F32, tag="S")
                                so = sr_offs[ci]
                                nc.vector.tensor_tensor(
                                    out=S[:, 0:n, :],
                                    in0=meta_t[:, so + pos:so + pos + n]
                                        .unsqueeze(2).to_broadcast(
                                            [P, n, P]),
                                    in1=iota16[:].unsqueeze(1)
                                        .to_broadcast([P, n, P]),
                                    op=mybir.AluOpType.is_equal)
                                for t in range(n):
                                    nc.tensor.matmul(
                                        out=ps[:, 0:D],
                                        lhsT=S[:, t, :],
                                        rhs=g_tiles[ci][:, pos + t, :],
                                        start=(tcount == 0),
                                        stop=(tcount == ntot - 1))
                                    tcount += 1
                            if phase == 1:
                                wr = w - s * SLW
                                nc.scalar.activation(
                                    stgE[:, wr * D:(wr + 1) * D],
                                    ps[:, 0:D], AF.Copy,
                                    scale=dv[:, w:w + 1])
                            else:
                                nc.scalar.activation(
                                    acc[:, w * D:(w + 1) * D],
                                    ps[:, 0:D], AF.Copy,
                                    scale=dv[:, w:w + 1])
                    if phase == 1:
                        sl3 = stgE[:].rearrange("p (w d) -> p w d", d=D)
                        nc.vector.tensor_tensor(
                            out=stgT[:].rearrange("p (w d) -> p w d", d=D),
                            in0=sl3,
                            in1=dv[:, s * SLW:(s + 1) * SLW].unsqueeze(2)
                                .to_broadcast([P, SLW, D]),
                            op=mybir.AluOpType.mult)
                        nc.sync.dma_start(
                            edges_o[k, s * SLR:(s + 1) * SLR, :]
                            .rearrange("(w p) d -> p w d", p=P),
                            stgE[:].rearrange("p (w d) -> p w d", d=D))
                        nc.sync.dma_start(
                            e_bounce[s * SLR:(s + 1) * SLR, :]
                            .rearrange("(w p) d -> p w d", p=P),
                            stgT[:].rearrange("p (w d) -> p w d", d=D))
                if phase == 1 and not SKIP_AG:
                    nc.gpsimd.collective_compute(
                        "AllGather", mybir.AluOpType.bypass,
                        replica_groups=RG,
                        ins=[e_bounce.ap().opt()],
                        outs=[e_full.ap().opt()])

            def h_slice(k_next, s, xsrc):
                """h table rows for slice s: dinv_{k_next}*(relu(x)+bias);
                write h_bounce[s], AllGather into h_full[s]."""
                dv = dinva[:, k_next * NT:(k_next + 1) * NT]
                for j0 in range(0, SLR, CCH):
                    cn = min(CCH, SLR - j0)
                    co = s * SLR + j0
                    h_s = po.tile([D, CCH], F32, tag="hx")
                    nc.sync.dma_start(h_s[:, 0:cn], xsrc[:, co:co + cn])
                    nc.scalar.activation(h_s[:, 0:cn], h_s[:, 0:cn],
                                         AF.Relu)
                    nc.vector.tensor_scalar_add(h_s[:, 0:cn], h_s[:, 0:cn],
                                                bcol[:, 0:1])
                    hr = stg.tile([P, (CCH // P) * D], F32, tag="hr")
                    for j in range(0, cn, P):
                        pj = min(P, cn - j)
                        w = (co + j) // P
                        pst = psh.tile([P, D], F32, tag="hT")
                        nc.tensor.transpose(pst[0:pj, :], h_s[:, j:j + pj],
                                            ident[0:D, 0:D])
                        jw = j // P
                        nc.scalar.activation(
                            hr[0:pj, jw * D:(jw + 1) * D], pst[0:pj, :],
                            AF.Copy, scale=dv[:, w:w + 1])
                    nc.sync.dma_start(
                        h_bounce[s * SLR + j0:s * SLR + j0 + cn, :]
                        .rearrange("(w p) d -> p w d", p=P),
                        hr[:, 0:(cn // P) * D]
                        .rearrange("p (w d) -> p w d", d=D))

            def ag_h():
                if SKIP_AG:
                    return
                nc.gpsimd.collective_compute(
                    "AllGather", mybir.AluOpType.bypass, replica_groups=RG,
                    ins=[h_bounce.ap().opt()],
                    outs=[h_full.ap().opt()])

            for s in range(NSL):
                h_slice(0, s, xT_pp[0])
            ag_h()

            for k in range(K):
                xcur = xT_pp[k % 2]
                xnxt = xT_pp[(k + 1) % 2]
                mo0 = mo[0]
                spmm(k, 1, h_full)
                mo[0] = mo0
                spmm(k, 2, e_full)

                # post phase: softmax -> fc1 -> fusion gate, per slice
                for s in range(NSL):
                    sl = acc[:, s * SLW * D:(s + 1) * SLW * D]
                    sl3 = sl.rearrange("p (w d) -> p w d", d=D)
                    nc.scalar.activation(sl, sl, AF.Exp)
                    ssum = po.tile([P, SLW], F32, tag="ssum")
                    nc.vector.reduce_sum(ssum[:, 0:SLW], sl3,
                                         axis=mybir.AxisListType.X)
                    nc.vector.reciprocal(ssum[:, 0:SLW], ssum[:, 0:SLW])
                    nc.vector.tensor_tensor(
                        out=sl3, in0=sl3,
                        in1=ssum[:, 0:SLW].unsqueeze(2)
                            .to_broadcast([P, SLW, D]),
                        op=mybir.AluOpType.mult)
                    for j0 in range(0, SLR, CCH):
                        cn = min(CCH, SLR - j0)
                        co = s * SLR + j0
                        psT = psp.tile([D, CCH], F32, tag="pp")
                        for j in range(0, cn, P):
                            pj = min(P, cn - j)
                            w = (co + j) // P
                            nc.tensor.transpose(
                                psT[:, j:j + pj],
                                acc[:, w * D:(w + 1) * D][0:pj, :],
                                ident[0:pj, 0:pj])
                        smT = po.tile([D, CCH], F32, tag="smT")
                        nc.scalar.activation(smT[:, 0:cn], psT[:, 0:cn],
                                             AF.Copy)
                        psN = psp.tile([D, CCH], F32, tag="pp")
                        nc.tensor.matmul(psN[:, 0:cn], lhsT=wfc1[:, :],
                                         rhs=smT[:, 0:cn], start=True,
                                         stop=True)
                        nodeT = po.tile([D, CCH], F32, tag="nodeT")
                        nc.scalar.activation(nodeT[:, 0:cn], psN[:, 0:cn],
                                             AF.Copy)
                        xc_s = po.tile([D, CCH], F32, tag="xc")
                        nc.sync.dma_start(xc_s[:, 0:cn],
                                          xcur[:, co:co + cn])
                        psG = psp.tile([D, CCH], F32, tag="pp")
                        nc.tensor.matmul(psG[:, 0:cn], lhsT=wfus[:, :],
                                         rhs=xc_s[:, 0:cn], start=True,
                                         stop=True)
                        t1x = po.tile([D, CCH], F32, tag="t1x")
                        nc.scalar.activation(t1x[:, 0:cn], psG[:, 0:cn],
                                             AF.Tanh, bias=b1c[:, 0:1])
                        psA0 = psp.tile([1, CCH], F32, tag="pp")
                        nc.tensor.matmul(psA0[:, 0:cn], lhsT=w2c[:, :],
                                         rhs=t1x[:, 0:cn], start=True,
                                         stop=True)
                        psG2 = psp.tile([D, CCH], F32, tag="pp")
                        nc.tensor.matmul(psG2[:, 0:cn], lhsT=wfus[:, :],
                                         rhs=nodeT[:, 0:cn], start=True,
                                         stop=True)
                        t1n = po.tile([D, CCH], F32, tag="t1x")
                        nc.scalar.activation(t1n[:, 0:cn], psG2[:, 0:cn],
                                             AF.Tanh, bias=b1c[:, 0:1])
                        psA1 = psp.tile([1, CCH], F32, tag="pp")
                        nc.tensor.matmul(psA1[:, 0:cn], lhsT=w2c[:, :],
                                         rhs=t1n[:, 0:cn], start=True,
                                         stop=True)
                        a1s = po.tile([1, CCH], F32, tag="a1s")
                        nc.scalar.activation(a1s[:, 0:cn], psA1[:, 0:cn],
                                             AF.Copy)
                        nc.vector.tensor_tensor(out=a1s[:, 0:cn],
                                                in0=psA0[:, 0:cn],
                                                in1=a1s[:, 0:cn],
                                                op=mybir.AluOpType.subtract)
                        nc.scalar.activation(a1s[:, 0:cn], a1s[:, 0:cn],
                                             AF.Sigmoid)
                        s0b = psp.tile([D, CCH], F32, tag="pp")
                        nc.tensor.matmul(s0b[:, 0:cn], lhsT=ones1[:, :],
                                         rhs=a1s[:, 0:cn], start=True,
                                         stop=True)
                        nc.vector.tensor_tensor(out=xc_s[:, 0:cn],
                                                in0=xc_s[:, 0:cn],
                                                in1=nodeT[:, 0:cn],
                                                op=mybir.AluOpType.subtract)
                        nc.vector.tensor_tensor(
                            out=xc_s[:, 0:cn], in0=xc_s[:, 0:cn],
                            in1=s0b[:, 0:cn],
                            op=mybir.AluOpType.mult)
                        nc.vector.tensor_tensor(out=nodeT[:, 0:cn],
                                                in0=nodeT[:, 0:cn],
                                                in1=xc_s[:, 0:cn],
                                                op=mybir.AluOpType.add)
                        nc.sync.dma_start(xnxt[:, co:co + cn],
                                          nodeT[:, 0:cn])
                        nc.sync.dma_start(nodesT_o[k, :, co:co + cn],
                                          nodeT[:, 0:cn])
                    if k < K - 1:
                        h_slice(k + 1, s, xnxt)
                if k < K - 1:
                    ag_h()
    nc.compile()
    return nc


_CACHE = {}


def _plan_key(cfg, plan):
    key = [cfg.NU, cfg.NI, cfg.K, cfg.E]
    for k in range(cfg.K):
        key.append(plan[k]["nt"].tobytes())
    return tuple(key)


def _get_nc(cfg, plan):
    key = _plan_key(cfg, plan)
    if key not in _CACHE:
        _CACHE[key] = build(cfg, plan)
    return _CACHE[key]


# build() needs plan in scope for _slice_meta_cols; stash it module-level
plan = None


def prepare(cfg, x, hgc1_bias, fc1_W, fus_l1_W, fus_l1_b, fus_l2_W, fus_l2_b,
            rows, cols):
    """Build (nc, in_maps, unshard) for this problem instance."""
    global plan
    x = np.asarray(x, np.float32)
    plan, per_core = prep(cfg, rows, cols)
    nc = _get_nc(cfg, plan)

    core_of, loc_of = _perm_maps(cfg)
    in_maps = []
    for m in range(NCORES):
        xm = np.zeros((cfg.SHP, D), np.float32)
        sel = core_of == m
        xm[loc_of[sel]] = x[sel]
        in_maps.append({
            "xT": np.ascontiguousarray(xm.T),
            "biascol": np.asarray(hgc1_bias, np.float32).reshape(D, 1),
            "fc1_WT": np.ascontiguousarray(np.asarray(fc1_W, np.float32).T),
            "fus1_WT": np.ascontiguousarray(
                np.asarray(fus_l1_W, np.float32).T),
            "b1col": np.asarray(fus_l1_b, np.float32).reshape(D, 1),
            "w2col": np.ascontiguousarray(
                np.asarray(fus_l2_W, np.float32).reshape(1, D).T),
            "meta": per_core[m]["meta"],
            "dinv": per_core[m]["dinv"],
        })

    def unshard(results):
        N = cfg.NU + cfg.NI
        nodes = np.zeros((cfg.K, N, D), np.float32)
        edges = np.zeros((cfg.K, N, D), np.float32)
        for m in range(NCORES):
            sel = core_of == m
            nodesT = np.asarray(results[m]["nodesT"]).reshape(
                cfg.K, D, cfg.SHP)
            edg = np.asarray(results[m]["edges"]).reshape(cfg.K, cfg.SHP, D)
            nodes[:, sel, :] = nodesT[:, :, loc_of[sel]].transpose(0, 2, 1)
            edges[:, sel, :] = edg[:, loc_of[sel], :]
        return nodes, edges

    return nc, in_maps, unshard


def run(cfg, x, hgc1_bias, fc1_W, fus_l1_W, fus_l1_b, fus_l2_W, fus_l2_b,
        rows, cols, sim=False):
    nc, in_maps, unshard = prepare(cfg, x, hgc1_bias, fc1_W, fus_l1_W,
                                   fus_l1_b, fus_l2_W, fus_l2_b, rows, cols)
    if sim:
        from concourse import bass_interp
        simu = bass_interp.MultiCoreSim(nc, NCORES)
        for m in range(NCORES):
            for kk, a in in_maps[m].items():
                simu.cores[m].tensor(kk)[:] = a
        simu.simulate()
        results = [{"nodesT": simu.cores[m].mem_tensor("nodesT"),
                    "edges": simu.cores[m].mem_tensor("edges")}
                   for m in range(NCORES)]
        exec_ns = None
    else:
        from concourse.bass_utils import run_bass_kernel_spmd
        res = run_bass_kernel_spmd(
            nc, in_maps, core_ids=list(range(NCORES)))
        results = res.results
        exec_ns = res.exec_time_ns

    return unshard(results), exec_ns


def kernel(x, hgc1_bias, fc1_W, fus_l1_W, fus_l1_b, fus_l2_W, fus_l2_b,
           rows, cols):
    cfg = Cfg(NU=50000, NI=50000, K=4, E=1000000)
    (nodes, edges), _ = run(cfg, x, hgc1_bias, fc1_W, fus_l1_W, fus_l1_b,
                            fus_l2_W, fus_l2_b, rows, cols)
    return nodes, edges
